# revision 23
# baseline (speedup 1.0000x reference)
"""Multi-head self-attention (B=2, S=2048, D=1024, H=16, causal) on 8 TRN2 cores.

Sharding: core c handles batch b=c//4 and head-group g=c%4 (4 heads each).
Host pre-transposes x and the weight slices so the kernel never needs an
on-chip transpose, and pre-converts them to bf16 (PE streams run at
1 cycle/row in bf16; DMA halves):
  xT   [1024, 2048] = x[b].T
  wqT/wkT/wvT [1024, 256] = W.T[:, g*256:(g+1)*256]
  woT  [256, 1024] = Wo[:, g*256:(g+1)*256].T
The kernel writes bf16 partial outputs; host sums the 4 per-group partials
per batch in fp32 at the end.

On-chip dataflow per core:
  qT/kT [256, 2048] (head dim on partitions), v [2048, 4*65] (with a ones
  column appended per head so the PV matmul also accumulates the softmax
  denominator in psum row 64).  Scores are computed transposed
  (scoresT[j, i]) so softmax needs no transpose at all; no max-subtraction
  (scores are O(+-6), exp is safe in fp32).

Performance notes (the PE tensor engine only reaches its ramped 2.4 GHz
p-state after ~3us of *continuous* work; any idle resets it to 1.2 GHz, so
everything is built around never letting the PE wait):
  - Attention runs a lag-2 software pipeline: scores(jc) are emitted two
    chunks ahead of PV(jc), so the PE has ~1.3us of score matmuls in its
    queue while the Scalar engine exponentiates a chunk.
  - Causal trimming: for the 4 diagonal j-chunks of each query block the
    scores / exp / PV are restricted to the query range [o*128, 512) that
    can actually attend to that chunk; the causal mask reduces to a single
    [128,128] lower-triangular multiply per diagonal chunk.
  - All PSUM tiles are one 2KB bank: a 6-deep "mm" pool (scores, QKV
    projections, out-proj) + 2 PV accumulators.
  - The output projection of block Q-1 is interleaved two sub-blocks at a
    time into the starts of block Q's two head-pair groups, filling the
    PE while the softmax denominators of the previous group are applied.
  - The softmax 1/denominator is broadcast across the 64 head-dim
    partitions on the (otherwise idle) GpSimd engine.
  - xT is DMAed in four column-quarters and the first projection group
    only needs the first quarter; DMA descriptor generation (~0.6us per
    transfer, serial per issuing queue) is kept off the critical path.
"""

import os
import sys

sys.path.insert(0, "/opt/trn_rl_repo")
os.environ.setdefault("MYCRO_LOCAL_CACHE", "1")

import numpy as np
import ml_dtypes

import concourse.bacc as bacc
import concourse.bass as bass
import concourse.mybir as mybir
import concourse.tile as tile
from concourse import bass_utils

# The agent image's antenv lacks axon_hooks, so bass_utils' trace path dies on
# import.  Register a shim module that lazily builds the ctypes NTFF hook.
if "antenv.axon_hooks" not in sys.modules:
    import types

    _shim = types.ModuleType("antenv.axon_hooks")
    _shim._HOOK = None

    def _set_hook(hook, _m=_shim):
        _m._HOOK = hook

    def _get_hook(_m=_shim):
        if _m._HOOK is None:
            try:
                from trn_agent_boot.trn_boot import _ntff_profile_via_ctypes

                _m._HOOK = _ntff_profile_via_ctypes("/opt/axon/libaxon_pjrt.so")
            except Exception:
                _m._HOOK = None
        return _m._HOOK

    _shim.set_axon_ntff_profile_hook = _set_hook
    _shim.get_axon_ntff_profile_hook = _get_hook
    sys.modules["antenv.axon_hooks"] = _shim

B, S, D, H = 2, 2048, 1024, 16
DK = 64                      # head dim
HC = 4                       # heads per core
GC = HC * DK                 # 256 cols per head-group
N_CORES = 8
SCALE = 1.0 / np.sqrt(DK)    # 0.125

F32 = mybir.dt.float32
BF16 = mybir.dt.bfloat16
NP_BF16 = ml_dtypes.bfloat16

TRACE = False
LAST_RESULTS = None


def build_bass():
    nc = bacc.Bacc("TRN2", target_bir_lowering=False, debug=False)

    xT_d = nc.dram_tensor("xT", [D, S], BF16, kind="ExternalInput")
    wqT_d = nc.dram_tensor("wqT", [D, GC], BF16, kind="ExternalInput")
    wkT_d = nc.dram_tensor("wkT", [D, GC], BF16, kind="ExternalInput")
    wvT_d = nc.dram_tensor("wvT", [D, GC], BF16, kind="ExternalInput")
    woT_d = nc.dram_tensor("woT", [GC, D], BF16, kind="ExternalInput")
    mask_d = nc.dram_tensor("mask", [128, 128], BF16, kind="ExternalInput")
    out_d = nc.dram_tensor("out", [S, D], BF16, kind="ExternalOutput")

    EXP = mybir.ActivationFunctionType.Exp

    with tile.TileContext(nc) as tc:
        with (
            nc.allow_low_precision(reason="bf16 matmuls, fp32 psum accumulate"),
            tc.tile_pool(name="const", bufs=1) as const,
            tc.tile_pool(name="work", bufs=6) as work,
            tc.tile_pool(name="apool", bufs=2) as apool,
            tc.tile_pool(name="opool", bufs=2) as opool,
            tc.tile_pool(name="rpool", bufs=2) as rpool,
            tc.tile_pool(name="pmm", bufs=4, space="PSUM") as pmm,
            tc.tile_pool(name="psout", bufs=4, space="PSUM") as psout,
        ):
            # ---- load inputs -------------------------------------------------
            xT_dr = xT_d.rearrange("(o p) s -> p o s", p=128)
            # one tile per s-quarter with exactly one DMA each: a reader of
            # quarter q then only waits for quarter q's transfer (a single
            # shared tile would make every reader wait for the whole 4MB)
            xqs = [const.tile([128, 8, 512], BF16, name=f"xq{q}")
                   for q in range(4)]
            # DMA queues are FIFO: whatever is enqueued first transfers
            # first.  The tiny q/k weights go ahead of the 4MB xT stream so
            # the first projection group (wq+wk+quarter 0) gates on ~1.3MB,
            # not on the whole input.  Descriptor generation (~1us each,
            # serial per issuing engine) runs on sync and gpsimd in parallel.
            wq = const.tile([128, 8, GC], BF16)
            nc.sync.dma_start(wq[:], wqT_d.rearrange("(o p) m -> p o m", p=128))
            wk = const.tile([128, 8, GC], BF16)
            nc.gpsimd.dma_start(wk[:], wkT_d.rearrange("(o p) m -> p o m", p=128))
            wv = const.tile([128, 8, GC], BF16)
            nc.gpsimd.dma_start(wv[:], wvT_d.rearrange("(o p) m -> p o m", p=128))
            for quarter in range(4):
                s0 = quarter * 512
                nc.sync.dma_start(
                    xqs[quarter][:], xT_dr[:, :, s0:s0 + 512]
                )
            wo = const.tile([128, 2, D], BF16)
            nc.gpsimd.dma_start(wo[:], woT_d.rearrange("(o p) n -> p o n", p=128))
            maskt = const.tile([128, 128], BF16)
            nc.gpsimd.dma_start(maskt[:], mask_d[:])

            ones_b = const.tile([128, 64], BF16)
            nc.vector.memset(ones_b[:], 1.0)

            # ---- projections -------------------------------------------------
            # qT/kT: per (head-pair mo, s-half sbh) tiles [128, 1024] so the
            # attention phase can start before all projections finish
            qts = [[const.tile([128, 1024], BF16, name=f"q{m}{s}")
                    for s in range(2)] for m in range(2)]
            kts = [[const.tile([128, 1024], BF16, name=f"k{m}{s}")
                    for s in range(2)] for m in range(2)]
            # v: per j-chunk tiles; per head: 64 value cols + 1 ones col
            vts = []
            for io in range(16):
                vt = const.tile([128, HC * 65], BF16, name=f"v{io}")
                nc.vector.tensor_copy(
                    vt.rearrange("p (h u) -> p h u", u=65)[:, :, 64],
                    ones_b[:, 0:4],
                )
                vts.append(vt)

            # s-quarter outer: each quarter's groups (q, k, and v) only gate
            # on that quarter's slice of the xT DMA, so the PE paces along
            # right behind the input stream
            for sb in range(4):
                for w_sb, dst in ((wq, qts), (wk, kts)):
                    for mo in range(2):
                        ps = pmm.tile([128, 512], F32, tag="mm")
                        for ko in range(8):
                            nc.tensor.matmul(
                                ps[:],
                                (w_sb[:, ko, mo * 128:(mo + 1) * 128]),
                                (xqs[sb][:, ko, :]),
                                start=(ko == 0),
                                stop=(ko == 7),
                                skip_group_check=True,
                            )
                        nc.vector.tensor_copy(
                            dst[mo][sb // 2][:, (sb % 2) * 512:(sb % 2 + 1) * 512],
                            ps[:],
                        )
                for io in range(4 * sb, 4 * sb + 4):
                    ps = pmm.tile([128, 256], F32, tag="mm")
                    for ko in range(8):
                        nc.tensor.matmul(
                            ps[:],
                            (xqs[sb][:, ko, (io % 4) * 128:(io % 4 + 1) * 128]),
                            (wv[:, ko, :]),
                            start=(ko == 0),
                            stop=(ko == 7),
                        )
                    nc.vector.tensor_copy(
                        vts[io].rearrange("p (h u) -> p h u", u=65)[:, :, 0:64],
                        ps.rearrange("p (h e) -> p h e", e=64),
                    )

            # ---- attention + output projection, per 512-query block ---------
            out_dr = out_d.rearrange("(a p) n -> p a n", p=128)

            def emit_outproj_chunk(Qprev, so, aTprev, split_dma=False):
                """partial[s, :] = a @ woT for query sub-block so of Qprev."""
                osb = opool.tile([128, D], BF16, tag="osb", name=f"osb{Qprev}{so}")
                for nt in range(2):
                    po = pmm.tile([128, 512], F32, tag="mm")
                    for co in range(2):
                        nc.tensor.matmul(
                            po[:],
                            (aTprev[:, co, so * 128:(so + 1) * 128]),
                            (wo[:, co, nt * 512:(nt + 1) * 512]),
                            start=(co == 0),
                            stop=(co == 1),
                            skip_group_check=True,
                        )
                    nc.vector.tensor_copy(
                        osb[:, nt * 512:(nt + 1) * 512], po[:]
                    )
                    if split_dma:  # tail: drain each half as soon as it's cast
                        nc.sync.dma_start(
                            out_dr[:, Qprev * 4 + so, nt * 512:(nt + 1) * 512],
                            osb[:, nt * 512:(nt + 1) * 512],
                        )
                if not split_dma:
                    nc.sync.dma_start(out_dr[:, Qprev * 4 + so, :], osb[:])

            LAG = 2
            aT_prev = None
            for Q in range(4):
                aT = apool.tile([128, 2, 512], BF16, tag="aT")
                for mo in range(2):
                    nchunks = (Q + 1) * 4
                    out_ps = [
                        psout.tile([65, 512], F32, tag="out", name=f"out_ps{_h}")
                        for _h in range(2)
                    ]
                    exs = {}

                    def chunk_lo(jc):
                        # diagonal chunks only see queries >= (jc-4Q)*128
                        return (jc - 4 * Q) * 128 if jc >= 4 * Q else 0

                    for jc in range(nchunks + LAG):
                        if jc < nchunks:
                            lo = chunk_lo(jc)
                            for hp in range(2):
                                sc = pmm.tile([128, 512], F32, tag="mm")
                                nc.tensor.matmul(
                                    sc[:, lo:512],
                                    (kts[mo][jc // 8][hp * 64:(hp + 1) * 64,
                                           (jc % 8) * 128:(jc % 8 + 1) * 128]),
                                    (qts[mo][Q // 2][hp * 64:(hp + 1) * 64,
                                           (Q % 2) * 512 + lo:(Q % 2 + 1) * 512]),
                                    start=True,
                                    stop=True,
                                    skip_group_check=True,
                                )
                                ex = work.tile([128, 512], BF16, tag="exp")
                                nc.scalar.activation(
                                    ex[:, lo:512], sc[:, lo:512], EXP, scale=SCALE
                                )
                                if lo > 0 or jc == 4 * Q:
                                    # triangular mask on the 128-col diag band
                                    nc.vector.tensor_mul(
                                        ex[:, lo:lo + 128],
                                        ex[:, lo:lo + 128],
                                        maskt[:],
                                    )
                                exs[(jc, hp)] = ex
                            # interleave previous block's output projection:
                            # two sub-blocks per mo group, at jc>=2 so they
                            # never head-of-line block this group's scores
                            if 2 <= jc < 4 and aT_prev is not None:
                                emit_outproj_chunk(Q - 1, 2 * mo + jc - 2, aT_prev)
                        jd = jc - LAG
                        if jd >= 0:
                            lo = chunk_lo(jd)
                            for hp in range(2):
                                h = 2 * mo + hp
                                nc.tensor.matmul(
                                    out_ps[hp][:, lo:512],
                                    (vts[jd][:, h * 65:(h + 1) * 65]),
                                    (exs.pop((jd, hp))[:, lo:512]),
                                    start=(jd == 0),
                                    stop=(jd == nchunks - 1),
                                    skip_group_check=True,
                                )
                    last_group = (Q == 3 and mo == 1)
                    if last_group:
                        # dummy matmuls keep the PE at its ramped p-state
                        # through the final normalize chain (the tiny copy
                        # gives them a reader so BIR verification passes)
                        for _w in range(2):
                            wps = pmm.tile([128, 512], F32, tag="mm")
                            nc.tensor.matmul(
                                wps[:],
                                (kts[0][0][0:64, 0:128]),
                                (qts[0][0][0:64, 0:512]),
                                start=True, stop=True, skip_group_check=True,
                            )
                            wsink = rpool.tile([1, 8], F32, tag="wsink")
                            nc.scalar.activation(
                                wsink[:], wps[0:1, 0:8], EXP, scale=0.0
                            )
                    for hp in range(2):
                        den = rpool.tile([1, 512], F32, tag="den")
                        nc.vector.tensor_copy(den[:], out_ps[hp][64:65, :])
                        rd_f = rpool.tile([1, 512], F32, tag="rdf")
                        nc.vector.reciprocal_approx_fast(out=rd_f[:], in_=den[:])
                        rd_b = rpool.tile([1, 512], BF16, tag="rdb16")
                        nc.vector.tensor_copy(rd_b[:], rd_f[:])
                        # broadcast 1/denom across the 64 head-dim partitions
                        # on GpSimd so the PE never sees a K=1 matmul
                        rdb = rpool.tile([64, 512], BF16, tag="rdb")
                        nc.gpsimd.partition_broadcast(rdb[:], rd_b[:])
                        nc.vector.tensor_mul(
                            aT[hp * 64:(hp + 1) * 64, mo, :],
                            out_ps[hp][0:64, :],
                            rdb[:],
                        )
                aT_prev = aT

            for so in range(4):  # last block's out-proj (nothing to hide it under)
                emit_outproj_chunk(3, so, aT_prev, split_dma=True)

    nc.compile()
    return nc


_NC = None


def _get_nc():
    global _NC
    if _NC is None:
        _NC = build_bass()
    return _NC


def _causal_mask():
    j = np.arange(128)[:, None]
    i = np.arange(128)[None, :]
    return (j <= i).astype(NP_BF16)


def kernel(in_features, Wq, Wk, Wv, Wo):
    global LAST_RESULTS
    nc = _get_nc()

    x = np.asarray(in_features, np.float32)
    Wq = np.asarray(Wq, np.float32)
    Wk = np.asarray(Wk, np.float32)
    Wv = np.asarray(Wv, np.float32)
    Wo = np.asarray(Wo, np.float32)
    mask = _causal_mask()

    in_maps = []
    for c in range(N_CORES):
        b, g = divmod(c, 4)
        cols = slice(g * GC, (g + 1) * GC)
        in_maps.append({
            "xT": np.ascontiguousarray(x[b].T).astype(NP_BF16),
            "wqT": np.ascontiguousarray(Wq.T[:, cols]).astype(NP_BF16),
            "wkT": np.ascontiguousarray(Wk.T[:, cols]).astype(NP_BF16),
            "wvT": np.ascontiguousarray(Wv.T[:, cols]).astype(NP_BF16),
            "woT": np.ascontiguousarray(Wo[:, cols].T).astype(NP_BF16),
            "mask": mask,
        })

    res = bass_utils.run_bass_kernel_spmd(
        nc, in_maps, core_ids=list(range(N_CORES)), trace=TRACE,
    )
    LAST_RESULTS = res
    parts = [res.results[c]["out"].astype(np.float32) for c in range(N_CORES)]
    out = np.stack([
        parts[4 * b] + parts[4 * b + 1] + parts[4 * b + 2] + parts[4 * b + 3]
        for b in range(B)
    ]).astype(np.float32)
    return out


# revision 25
# speedup vs baseline: 1.0293x; 1.0293x over previous
"""Multi-head self-attention (B=2, S=2048, D=1024, H=16, causal) on 8 TRN2 cores.

Sharding: core c handles batch b=c//4 and head-group g=c%4 (4 heads each).
Host pre-transposes x and the weight slices so the kernel never needs an
on-chip transpose, and pre-converts them to bf16 (PE streams run at
1 cycle/row in bf16; DMA halves):
  xT   [1024, 2048] = x[b].T
  wqT/wkT/wvT [1024, 256] = W.T[:, g*256:(g+1)*256]
  woT  [256, 1024] = Wo[:, g*256:(g+1)*256].T
The kernel writes bf16 partial outputs; host sums the 4 per-group partials
per batch in fp32 at the end.

On-chip dataflow per core:
  qT/kT [256, 2048] (head dim on partitions), v [2048, 4*65] (with a ones
  column appended per head so the PV matmul also accumulates the softmax
  denominator in psum row 64).  Scores are computed transposed
  (scoresT[j, i]) so softmax needs no transpose at all; no max-subtraction
  (scores are O(+-6), exp is safe in fp32).

Performance notes (the PE tensor engine only reaches its ramped 2.4 GHz
p-state after ~3us of *continuous* work; any idle resets it to 1.2 GHz, so
everything is built around never letting the PE wait):
  - Attention runs a lag-2 software pipeline: scores(jc) are emitted two
    chunks ahead of PV(jc), so the PE has ~1.3us of score matmuls in its
    queue while the Scalar engine exponentiates a chunk.
  - Causal trimming: for the 4 diagonal j-chunks of each query block the
    scores / exp / PV are restricted to the query range [o*128, 512) that
    can actually attend to that chunk; the causal mask reduces to a single
    [128,128] lower-triangular multiply per diagonal chunk.
  - All PSUM tiles are one 2KB bank: a 6-deep "mm" pool (scores, QKV
    projections, out-proj) + 2 PV accumulators.
  - The output projection of block Q-1 is interleaved two sub-blocks at a
    time into the starts of block Q's two head-pair groups, filling the
    PE while the softmax denominators of the previous group are applied.
  - The softmax 1/denominator is broadcast across the 64 head-dim
    partitions on the (otherwise idle) GpSimd engine.
  - xT is DMAed in four column-quarters and the first projection group
    only needs the first quarter; DMA descriptor generation (~0.6us per
    transfer, serial per issuing queue) is kept off the critical path.
"""

import os
import sys

sys.path.insert(0, "/opt/trn_rl_repo")
os.environ.setdefault("MYCRO_LOCAL_CACHE", "1")

import numpy as np
import ml_dtypes

import concourse.bacc as bacc
import concourse.bass as bass
import concourse.mybir as mybir
import concourse.tile as tile
from concourse import bass_utils

# The agent image's antenv lacks axon_hooks, so bass_utils' trace path dies on
# import.  Register a shim module that lazily builds the ctypes NTFF hook.
if "antenv.axon_hooks" not in sys.modules:
    import types

    _shim = types.ModuleType("antenv.axon_hooks")
    _shim._HOOK = None

    def _set_hook(hook, _m=_shim):
        _m._HOOK = hook

    def _get_hook(_m=_shim):
        if _m._HOOK is None:
            try:
                from trn_agent_boot.trn_boot import _ntff_profile_via_ctypes

                _m._HOOK = _ntff_profile_via_ctypes("/opt/axon/libaxon_pjrt.so")
            except Exception:
                _m._HOOK = None
        return _m._HOOK

    _shim.set_axon_ntff_profile_hook = _set_hook
    _shim.get_axon_ntff_profile_hook = _get_hook
    sys.modules["antenv.axon_hooks"] = _shim

B, S, D, H = 2, 2048, 1024, 16
DK = 64                      # head dim
HC = 4                       # heads per core
GC = HC * DK                 # 256 cols per head-group
N_CORES = 8
SCALE = 1.0 / np.sqrt(DK)    # 0.125

F32 = mybir.dt.float32
BF16 = mybir.dt.bfloat16
NP_BF16 = ml_dtypes.bfloat16

TRACE = False
LAST_RESULTS = None


def build_bass():
    nc = bacc.Bacc("TRN2", target_bir_lowering=False, debug=False)

    xT_d = nc.dram_tensor("xT", [D, S], BF16, kind="ExternalInput")
    wqT_d = nc.dram_tensor("wqT", [D, GC], BF16, kind="ExternalInput")
    wkT_d = nc.dram_tensor("wkT", [D, GC], BF16, kind="ExternalInput")
    wvT_d = nc.dram_tensor("wvT", [D, GC], BF16, kind="ExternalInput")
    woT_d = nc.dram_tensor("woT", [GC, D], BF16, kind="ExternalInput")
    mask_d = nc.dram_tensor("mask", [128, 128], BF16, kind="ExternalInput")
    out_d = nc.dram_tensor("out", [S, D], BF16, kind="ExternalOutput")

    EXP = mybir.ActivationFunctionType.Exp

    with tile.TileContext(nc) as tc:
        with (
            nc.allow_low_precision(reason="bf16 matmuls, fp32 psum accumulate"),
            tc.tile_pool(name="const", bufs=1) as const,
            tc.tile_pool(name="work", bufs=6) as work,
            tc.tile_pool(name="apool", bufs=2) as apool,
            tc.tile_pool(name="opool", bufs=2) as opool,
            tc.tile_pool(name="rpool", bufs=2) as rpool,
            tc.tile_pool(name="pmm", bufs=4, space="PSUM") as pmm,
            tc.tile_pool(name="psout", bufs=4, space="PSUM") as psout,
        ):
            # ---- load inputs -------------------------------------------------
            xT_dr = xT_d.rearrange("(o p) s -> p o s", p=128)
            # one tile per s-quarter with exactly one DMA each: a reader of
            # quarter q then only waits for quarter q's transfer (a single
            # shared tile would make every reader wait for the whole 4MB)
            xqs = [const.tile([128, 8, 512], BF16, name=f"xq{q}")
                   for q in range(4)]
            # DMA queues are FIFO and an instruction's coalesced semaphore
            # wait covers every DMA issued before it in program order, so
            # loads are emitted in strict dependency order, interleaved with
            # the compute that consumes them (later quarters are issued from
            # inside the projection loop below).  The tiny q/k weights go
            # ahead of the 4MB xT stream so the first projection group gates
            # on ~1.3MB, not the whole input.
            wq = const.tile([128, 8, GC], BF16)
            nc.sync.dma_start(wq[:], wqT_d.rearrange("(o p) m -> p o m", p=128))
            wk = const.tile([128, 8, GC], BF16)
            nc.gpsimd.dma_start(wk[:], wkT_d.rearrange("(o p) m -> p o m", p=128))
            nc.sync.dma_start(xqs[0][:], xT_dr[:, :, 0:512])
            wv = const.tile([128, 8, GC], BF16)
            nc.gpsimd.dma_start(wv[:], wvT_d.rearrange("(o p) m -> p o m", p=128))
            wo = const.tile([128, 2, D], BF16)
            maskt = const.tile([128, 128], BF16)

            ones_b = const.tile([128, 64], BF16)
            nc.vector.memset(ones_b[:], 1.0)

            # ---- projections -------------------------------------------------
            # qT/kT: per (head-pair mo, s-half sbh) tiles [128, 1024] so the
            # attention phase can start before all projections finish
            qts = [[const.tile([128, 1024], BF16, name=f"q{m}{s}")
                    for s in range(2)] for m in range(2)]
            kts = [[const.tile([128, 1024], BF16, name=f"k{m}{s}")
                    for s in range(2)] for m in range(2)]
            # v: per j-chunk tiles; per head: 64 value cols + 1 ones col
            vts = []
            for io in range(16):
                vt = const.tile([128, HC * 65], BF16, name=f"v{io}")
                nc.vector.tensor_copy(
                    vt.rearrange("p (h u) -> p h u", u=65)[:, :, 64],
                    ones_b[:, 0:4],
                )
                vts.append(vt)

            # s-quarter outer: each quarter's groups (q, k, and v) only gate
            # on that quarter's slice of the xT DMA, so the PE paces along
            # right behind the input stream
            for sb in range(4):
                if sb < 3:  # issue the next quarter's load behind this one
                    s0 = (sb + 1) * 512
                    nc.sync.dma_start(xqs[sb + 1][:], xT_dr[:, :, s0:s0 + 512])
                if sb == 1:
                    nc.gpsimd.dma_start(
                        wo[:], woT_d.rearrange("(o p) n -> p o n", p=128)
                    )
                    nc.gpsimd.dma_start(maskt[:], mask_d[:])
                for w_sb, dst in ((wq, qts), (wk, kts)):
                    for mo in range(2):
                        ps = pmm.tile([128, 512], F32, tag="mm")
                        for ko in range(8):
                            nc.tensor.matmul(
                                ps[:],
                                (w_sb[:, ko, mo * 128:(mo + 1) * 128]),
                                (xqs[sb][:, ko, :]),
                                start=(ko == 0),
                                stop=(ko == 7),
                                skip_group_check=True,
                            )
                        nc.vector.tensor_copy(
                            dst[mo][sb // 2][:, (sb % 2) * 512:(sb % 2 + 1) * 512],
                            ps[:],
                        )
                for io in range(4 * sb, 4 * sb + 4):
                    ps = pmm.tile([128, 256], F32, tag="mm")
                    for ko in range(8):
                        nc.tensor.matmul(
                            ps[:],
                            (xqs[sb][:, ko, (io % 4) * 128:(io % 4 + 1) * 128]),
                            (wv[:, ko, :]),
                            start=(ko == 0),
                            stop=(ko == 7),
                        )
                    nc.vector.tensor_copy(
                        vts[io].rearrange("p (h u) -> p h u", u=65)[:, :, 0:64],
                        ps.rearrange("p (h e) -> p h e", e=64),
                    )

            # ---- attention + output projection, per 512-query block ---------
            out_dr = out_d.rearrange("(a p) n -> p a n", p=128)

            def emit_outproj_chunk(Qprev, so, aTprev, split_dma=False):
                """partial[s, :] = a @ woT for query sub-block so of Qprev."""
                osb = opool.tile([128, D], BF16, tag="osb", name=f"osb{Qprev}{so}")
                for nt in range(2):
                    po = pmm.tile([128, 512], F32, tag="mm")
                    for co in range(2):
                        nc.tensor.matmul(
                            po[:],
                            (aTprev[:, co, so * 128:(so + 1) * 128]),
                            (wo[:, co, nt * 512:(nt + 1) * 512]),
                            start=(co == 0),
                            stop=(co == 1),
                            skip_group_check=True,
                        )
                    nc.vector.tensor_copy(
                        osb[:, nt * 512:(nt + 1) * 512], po[:]
                    )
                    if split_dma:  # tail: drain each half as soon as it's cast
                        nc.sync.dma_start(
                            out_dr[:, Qprev * 4 + so, nt * 512:(nt + 1) * 512],
                            osb[:, nt * 512:(nt + 1) * 512],
                        )
                if not split_dma:
                    nc.sync.dma_start(out_dr[:, Qprev * 4 + so, :], osb[:])

            LAG = 2
            aT_prev = None
            for Q in range(4):
                aT = apool.tile([128, 2, 512], BF16, tag="aT")
                for mo in range(2):
                    nchunks = (Q + 1) * 4
                    out_ps = [
                        psout.tile([65, 512], F32, tag="out", name=f"out_ps{_h}")
                        for _h in range(2)
                    ]
                    exs = {}

                    def chunk_lo(jc):
                        # diagonal chunks only see queries >= (jc-4Q)*128
                        return (jc - 4 * Q) * 128 if jc >= 4 * Q else 0

                    for jc in range(nchunks + LAG):
                        if jc < nchunks:
                            lo = chunk_lo(jc)
                            for hp in range(2):
                                sc = pmm.tile([128, 512], F32, tag="mm")
                                nc.tensor.matmul(
                                    sc[:, lo:512],
                                    (kts[mo][jc // 8][hp * 64:(hp + 1) * 64,
                                           (jc % 8) * 128:(jc % 8 + 1) * 128]),
                                    (qts[mo][Q // 2][hp * 64:(hp + 1) * 64,
                                           (Q % 2) * 512 + lo:(Q % 2 + 1) * 512]),
                                    start=True,
                                    stop=True,
                                    skip_group_check=True,
                                )
                                ex = work.tile([128, 512], BF16, tag="exp")
                                nc.scalar.activation(
                                    ex[:, lo:512], sc[:, lo:512], EXP, scale=SCALE
                                )
                                if lo > 0 or jc == 4 * Q:
                                    # triangular mask on the 128-col diag band
                                    nc.vector.tensor_mul(
                                        ex[:, lo:lo + 128],
                                        ex[:, lo:lo + 128],
                                        maskt[:],
                                    )
                                exs[(jc, hp)] = ex
                            # interleave previous block's output projection:
                            # two sub-blocks per mo group, at jc>=2 so they
                            # never head-of-line block this group's scores
                            if 2 <= jc < 4 and aT_prev is not None:
                                emit_outproj_chunk(Q - 1, 2 * mo + jc - 2, aT_prev)
                        jd = jc - LAG
                        if jd >= 0:
                            lo = chunk_lo(jd)
                            for hp in range(2):
                                h = 2 * mo + hp
                                nc.tensor.matmul(
                                    out_ps[hp][:, lo:512],
                                    (vts[jd][:, h * 65:(h + 1) * 65]),
                                    (exs.pop((jd, hp))[:, lo:512]),
                                    start=(jd == 0),
                                    stop=(jd == nchunks - 1),
                                    skip_group_check=True,
                                )
                    last_group = (Q == 3 and mo == 1)
                    if last_group:
                        # dummy matmuls keep the PE at its ramped p-state
                        # through the final normalize chain (the tiny copy
                        # gives them a reader so BIR verification passes)
                        for _w in range(2):
                            wps = pmm.tile([128, 512], F32, tag="mm")
                            nc.tensor.matmul(
                                wps[:],
                                (kts[0][0][0:64, 0:128]),
                                (qts[0][0][0:64, 0:512]),
                                start=True, stop=True, skip_group_check=True,
                            )
                            wsink = rpool.tile([1, 8], F32, tag="wsink")
                            nc.scalar.activation(
                                wsink[:], wps[0:1, 0:8], EXP, scale=0.0
                            )
                    for hp in range(2):
                        den = rpool.tile([1, 512], F32, tag="den")
                        nc.vector.tensor_copy(den[:], out_ps[hp][64:65, :])
                        rd_f = rpool.tile([1, 512], F32, tag="rdf")
                        nc.vector.reciprocal_approx_fast(out=rd_f[:], in_=den[:])
                        rd_b = rpool.tile([1, 512], BF16, tag="rdb16")
                        nc.vector.tensor_copy(rd_b[:], rd_f[:])
                        # broadcast 1/denom across the 64 head-dim partitions
                        # on GpSimd so the PE never sees a K=1 matmul
                        rdb = rpool.tile([64, 512], BF16, tag="rdb")
                        nc.gpsimd.partition_broadcast(rdb[:], rd_b[:])
                        nc.vector.tensor_mul(
                            aT[hp * 64:(hp + 1) * 64, mo, :],
                            out_ps[hp][0:64, :],
                            rdb[:],
                        )
                aT_prev = aT

            for so in range(4):  # last block's out-proj (nothing to hide it under)
                emit_outproj_chunk(3, so, aT_prev, split_dma=True)

    nc.compile()
    return nc


_NC = None


def _get_nc():
    global _NC
    if _NC is None:
        _NC = build_bass()
    return _NC


def _causal_mask():
    j = np.arange(128)[:, None]
    i = np.arange(128)[None, :]
    return (j <= i).astype(NP_BF16)


def kernel(in_features, Wq, Wk, Wv, Wo):
    global LAST_RESULTS
    nc = _get_nc()

    x = np.asarray(in_features, np.float32)
    Wq = np.asarray(Wq, np.float32)
    Wk = np.asarray(Wk, np.float32)
    Wv = np.asarray(Wv, np.float32)
    Wo = np.asarray(Wo, np.float32)
    mask = _causal_mask()

    in_maps = []
    for c in range(N_CORES):
        b, g = divmod(c, 4)
        cols = slice(g * GC, (g + 1) * GC)
        in_maps.append({
            "xT": np.ascontiguousarray(x[b].T).astype(NP_BF16),
            "wqT": np.ascontiguousarray(Wq.T[:, cols]).astype(NP_BF16),
            "wkT": np.ascontiguousarray(Wk.T[:, cols]).astype(NP_BF16),
            "wvT": np.ascontiguousarray(Wv.T[:, cols]).astype(NP_BF16),
            "woT": np.ascontiguousarray(Wo[:, cols].T).astype(NP_BF16),
            "mask": mask,
        })

    res = bass_utils.run_bass_kernel_spmd(
        nc, in_maps, core_ids=list(range(N_CORES)), trace=TRACE,
    )
    LAST_RESULTS = res
    parts = [res.results[c]["out"].astype(np.float32) for c in range(N_CORES)]
    out = np.stack([
        parts[4 * b] + parts[4 * b + 1] + parts[4 * b + 2] + parts[4 * b + 3]
        for b in range(B)
    ]).astype(np.float32)
    return out


# revision 26
# speedup vs baseline: 1.0471x; 1.0173x over previous
"""Multi-head self-attention (B=2, S=2048, D=1024, H=16, causal) on 8 TRN2 cores.

Sharding: core c handles batch b=c//4 and head-group g=c%4 (4 heads each).
Host pre-transposes x and the weight slices so the kernel never needs an
on-chip transpose, and pre-converts them to bf16 (PE streams run at
1 cycle/row in bf16; DMA halves):
  xT   [1024, 2048] = x[b].T
  wqT/wkT/wvT [1024, 256] = W.T[:, g*256:(g+1)*256]
  woT  [256, 1024] = Wo[:, g*256:(g+1)*256].T
The kernel writes bf16 partial outputs; host sums the 4 per-group partials
per batch in fp32 at the end.

On-chip dataflow per core:
  qT/kT [256, 2048] (head dim on partitions), v [2048, 4*65] (with a ones
  column appended per head so the PV matmul also accumulates the softmax
  denominator in psum row 64).  Scores are computed transposed
  (scoresT[j, i]) so softmax needs no transpose at all; no max-subtraction
  (scores are O(+-6), exp is safe in fp32).

Performance notes (the PE tensor engine only reaches its ramped 2.4 GHz
p-state after ~3us of *continuous* work; any idle resets it to 1.2 GHz, so
everything is built around never letting the PE wait):
  - Attention runs a lag-2 software pipeline: scores(jc) are emitted two
    chunks ahead of PV(jc), so the PE has ~1.3us of score matmuls in its
    queue while the Scalar engine exponentiates a chunk.
  - Causal trimming: for the 4 diagonal j-chunks of each query block the
    scores / exp / PV are restricted to the query range [o*128, 512) that
    can actually attend to that chunk; the causal mask reduces to a single
    [128,128] lower-triangular multiply per diagonal chunk.
  - All PSUM tiles are one 2KB bank: a 6-deep "mm" pool (scores, QKV
    projections, out-proj) + 2 PV accumulators.
  - The output projection of block Q-1 is interleaved two sub-blocks at a
    time into the starts of block Q's two head-pair groups, filling the
    PE while the softmax denominators of the previous group are applied.
  - The softmax 1/denominator is broadcast across the 64 head-dim
    partitions on the (otherwise idle) GpSimd engine.
  - xT is DMAed in four column-quarters and the first projection group
    only needs the first quarter; DMA descriptor generation (~0.6us per
    transfer, serial per issuing queue) is kept off the critical path.
"""

import os
import sys

sys.path.insert(0, "/opt/trn_rl_repo")
os.environ.setdefault("MYCRO_LOCAL_CACHE", "1")

import numpy as np
import ml_dtypes

import concourse.bacc as bacc
import concourse.bass as bass
import concourse.mybir as mybir
import concourse.tile as tile
from concourse import bass_utils

# The agent image's antenv lacks axon_hooks, so bass_utils' trace path dies on
# import.  Register a shim module that lazily builds the ctypes NTFF hook.
if "antenv.axon_hooks" not in sys.modules:
    import types

    _shim = types.ModuleType("antenv.axon_hooks")
    _shim._HOOK = None

    def _set_hook(hook, _m=_shim):
        _m._HOOK = hook

    def _get_hook(_m=_shim):
        if _m._HOOK is None:
            try:
                from trn_agent_boot.trn_boot import _ntff_profile_via_ctypes

                _m._HOOK = _ntff_profile_via_ctypes("/opt/axon/libaxon_pjrt.so")
            except Exception:
                _m._HOOK = None
        return _m._HOOK

    _shim.set_axon_ntff_profile_hook = _set_hook
    _shim.get_axon_ntff_profile_hook = _get_hook
    sys.modules["antenv.axon_hooks"] = _shim

B, S, D, H = 2, 2048, 1024, 16
DK = 64                      # head dim
HC = 4                       # heads per core
GC = HC * DK                 # 256 cols per head-group
N_CORES = 8
SCALE = 1.0 / np.sqrt(DK)    # 0.125

F32 = mybir.dt.float32
BF16 = mybir.dt.bfloat16
NP_BF16 = ml_dtypes.bfloat16

TRACE = False
LAST_RESULTS = None


def build_bass():
    nc = bacc.Bacc("TRN2", target_bir_lowering=False, debug=False)

    xT_d = nc.dram_tensor("xT", [D, S], BF16, kind="ExternalInput")
    wqT_d = nc.dram_tensor("wqT", [D, GC], BF16, kind="ExternalInput")
    wkT_d = nc.dram_tensor("wkT", [D, GC], BF16, kind="ExternalInput")
    wvT_d = nc.dram_tensor("wvT", [D, GC], BF16, kind="ExternalInput")
    woT_d = nc.dram_tensor("woT", [GC, D], BF16, kind="ExternalInput")
    mask_d = nc.dram_tensor("mask", [128, 128], BF16, kind="ExternalInput")
    out_d = nc.dram_tensor("out", [S, D], BF16, kind="ExternalOutput")

    EXP = mybir.ActivationFunctionType.Exp

    with tile.TileContext(nc) as tc:
        with (
            nc.allow_low_precision(reason="bf16 matmuls, fp32 psum accumulate"),
            tc.tile_pool(name="const", bufs=1) as const,
            tc.tile_pool(name="work", bufs=6) as work,
            tc.tile_pool(name="apool", bufs=2) as apool,
            tc.tile_pool(name="opool", bufs=2) as opool,
            tc.tile_pool(name="rpool", bufs=2) as rpool,
            tc.tile_pool(name="pmm", bufs=4, space="PSUM") as pmm,
            tc.tile_pool(name="psout", bufs=4, space="PSUM") as psout,
        ):
            # ---- load inputs -------------------------------------------------
            xT_dr = xT_d.rearrange("(o p) s -> p o s", p=128)
            # one tile per s-quarter with exactly one DMA each: a reader of
            # quarter q then only waits for quarter q's transfer (a single
            # shared tile would make every reader wait for the whole 4MB)
            xqs = [const.tile([128, 8, 512], BF16, name=f"xq{q}")
                   for q in range(4)]
            # DMA queues are FIFO and an instruction's coalesced semaphore
            # wait covers every DMA issued before it in program order, so
            # loads are emitted in strict dependency order, interleaved with
            # the compute that consumes them (later quarters are issued from
            # inside the projection loop below).  The tiny q/k weights go
            # ahead of the 4MB xT stream so the first projection group gates
            # on ~1.3MB, not the whole input.
            nc.sync.dma_start(xqs[0][:], xT_dr[:, :, 0:512])
            wq = const.tile([128, 8, GC], BF16)
            nc.sync.dma_start(wq[:], wqT_d.rearrange("(o p) m -> p o m", p=128))
            wk = const.tile([128, 8, GC], BF16)
            nc.gpsimd.dma_start(wk[:], wkT_d.rearrange("(o p) m -> p o m", p=128))
            wv = const.tile([128, 8, GC], BF16)
            nc.gpsimd.dma_start(wv[:], wvT_d.rearrange("(o p) m -> p o m", p=128))
            wo = const.tile([128, 2, D], BF16)
            maskt = const.tile([128, 128], BF16)

            ones_b = const.tile([128, 64], BF16)
            nc.vector.memset(ones_b[:], 1.0)

            # ---- projections -------------------------------------------------
            # qT/kT: per (head-pair mo, s-half sbh) tiles [128, 1024] so the
            # attention phase can start before all projections finish
            qts = [[const.tile([128, 1024], BF16, name=f"q{m}{s}")
                    for s in range(2)] for m in range(2)]
            kts = [[const.tile([128, 1024], BF16, name=f"k{m}{s}")
                    for s in range(2)] for m in range(2)]
            # v: per j-chunk tiles; per head: 64 value cols + 1 ones col
            vts = []
            for io in range(16):
                vt = const.tile([128, HC * 65], BF16, name=f"v{io}")
                nc.vector.tensor_copy(
                    vt.rearrange("p (h u) -> p h u", u=65)[:, :, 64],
                    ones_b[:, 0:4],
                )
                vts.append(vt)

            # s-quarter outer: each quarter's groups (q, k, and v) only gate
            # on that quarter's slice of the xT DMA, so the PE paces along
            # right behind the input stream
            for sb in range(4):
                if sb < 3:  # issue the next quarter's load behind this one
                    s0 = (sb + 1) * 512
                    nc.sync.dma_start(xqs[sb + 1][:], xT_dr[:, :, s0:s0 + 512])
                if sb == 1:
                    nc.gpsimd.dma_start(
                        wo[:], woT_d.rearrange("(o p) n -> p o n", p=128)
                    )
                    nc.gpsimd.dma_start(maskt[:], mask_d[:])
                for w_sb, dst in ((wq, qts), (wk, kts)):
                    for mo in range(2):
                        ps = pmm.tile([128, 512], F32, tag="mm")
                        for ko in range(8):
                            nc.tensor.matmul(
                                ps[:],
                                (w_sb[:, ko, mo * 128:(mo + 1) * 128]),
                                (xqs[sb][:, ko, :]),
                                start=(ko == 0),
                                stop=(ko == 7),
                                skip_group_check=True,
                            )
                        nc.vector.tensor_copy(
                            dst[mo][sb // 2][:, (sb % 2) * 512:(sb % 2 + 1) * 512],
                            ps[:],
                        )
                for io in range(4 * sb, 4 * sb + 4):
                    ps = pmm.tile([128, 256], F32, tag="mm")
                    for ko in range(8):
                        nc.tensor.matmul(
                            ps[:],
                            (xqs[sb][:, ko, (io % 4) * 128:(io % 4 + 1) * 128]),
                            (wv[:, ko, :]),
                            start=(ko == 0),
                            stop=(ko == 7),
                        )
                    nc.vector.tensor_copy(
                        vts[io].rearrange("p (h u) -> p h u", u=65)[:, :, 0:64],
                        ps.rearrange("p (h e) -> p h e", e=64),
                    )

            # ---- attention + output projection, per 512-query block ---------
            out_dr = out_d.rearrange("(a p) n -> p a n", p=128)

            def emit_outproj_chunk(Qprev, so, aTprev, split_dma=False):
                """partial[s, :] = a @ woT for query sub-block so of Qprev."""
                osb = opool.tile([128, D], BF16, tag="osb", name=f"osb{Qprev}{so}")
                for nt in range(2):
                    po = pmm.tile([128, 512], F32, tag="mm")
                    for co in range(2):
                        nc.tensor.matmul(
                            po[:],
                            (aTprev[:, co, so * 128:(so + 1) * 128]),
                            (wo[:, co, nt * 512:(nt + 1) * 512]),
                            start=(co == 0),
                            stop=(co == 1),
                            skip_group_check=True,
                        )
                    nc.vector.tensor_copy(
                        osb[:, nt * 512:(nt + 1) * 512], po[:]
                    )
                    if split_dma:  # tail: drain each half as soon as it's cast
                        nc.sync.dma_start(
                            out_dr[:, Qprev * 4 + so, nt * 512:(nt + 1) * 512],
                            osb[:, nt * 512:(nt + 1) * 512],
                        )
                if not split_dma:
                    nc.sync.dma_start(out_dr[:, Qprev * 4 + so, :], osb[:])

            LAG = 2
            aT_prev = None
            for Q in range(4):
                aT = apool.tile([128, 2, 512], BF16, tag="aT")
                for mo in range(2):
                    nchunks = (Q + 1) * 4
                    out_ps = [
                        psout.tile([65, 512], F32, tag="out", name=f"out_ps{_h}")
                        for _h in range(2)
                    ]
                    exs = {}

                    def chunk_lo(jc):
                        # diagonal chunks only see queries >= (jc-4Q)*128
                        return (jc - 4 * Q) * 128 if jc >= 4 * Q else 0

                    for jc in range(nchunks + LAG):
                        if jc < nchunks:
                            lo = chunk_lo(jc)
                            for hp in range(2):
                                sc = pmm.tile([128, 512], F32, tag="mm")
                                nc.tensor.matmul(
                                    sc[:, lo:512],
                                    (kts[mo][jc // 8][hp * 64:(hp + 1) * 64,
                                           (jc % 8) * 128:(jc % 8 + 1) * 128]),
                                    (qts[mo][Q // 2][hp * 64:(hp + 1) * 64,
                                           (Q % 2) * 512 + lo:(Q % 2 + 1) * 512]),
                                    start=True,
                                    stop=True,
                                    skip_group_check=True,
                                )
                                ex = work.tile([128, 512], BF16, tag="exp")
                                nc.scalar.activation(
                                    ex[:, lo:512], sc[:, lo:512], EXP, scale=SCALE
                                )
                                if lo > 0 or jc == 4 * Q:
                                    # triangular mask on the 128-col diag band
                                    nc.vector.tensor_mul(
                                        ex[:, lo:lo + 128],
                                        ex[:, lo:lo + 128],
                                        maskt[:],
                                    )
                                exs[(jc, hp)] = ex
                            # interleave previous block's output projection:
                            # two sub-blocks per mo group, at jc>=2 so they
                            # never head-of-line block this group's scores
                            if 2 <= jc < 4 and aT_prev is not None:
                                emit_outproj_chunk(Q - 1, 2 * mo + jc - 2, aT_prev)
                        jd = jc - LAG
                        if jd >= 0:
                            lo = chunk_lo(jd)
                            for hp in range(2):
                                h = 2 * mo + hp
                                nc.tensor.matmul(
                                    out_ps[hp][:, lo:512],
                                    (vts[jd][:, h * 65:(h + 1) * 65]),
                                    (exs.pop((jd, hp))[:, lo:512]),
                                    start=(jd == 0),
                                    stop=(jd == nchunks - 1),
                                    skip_group_check=True,
                                )
                    last_group = (Q == 3 and mo == 1)
                    if last_group:
                        # dummy matmuls keep the PE at its ramped p-state
                        # through the final normalize chain (the tiny copy
                        # gives them a reader so BIR verification passes)
                        for _w in range(2):
                            wps = pmm.tile([128, 512], F32, tag="mm")
                            nc.tensor.matmul(
                                wps[:],
                                (kts[0][0][0:64, 0:128]),
                                (qts[0][0][0:64, 0:512]),
                                start=True, stop=True, skip_group_check=True,
                            )
                            wsink = rpool.tile([1, 8], F32, tag="wsink")
                            nc.scalar.activation(
                                wsink[:], wps[0:1, 0:8], EXP, scale=0.0
                            )
                    for hp in range(2):
                        den = rpool.tile([1, 512], F32, tag="den")
                        nc.vector.tensor_copy(den[:], out_ps[hp][64:65, :])
                        rd_f = rpool.tile([1, 512], F32, tag="rdf")
                        nc.vector.reciprocal_approx_fast(out=rd_f[:], in_=den[:])
                        rd_b = rpool.tile([1, 512], BF16, tag="rdb16")
                        nc.vector.tensor_copy(rd_b[:], rd_f[:])
                        # broadcast 1/denom across the 64 head-dim partitions
                        # on GpSimd so the PE never sees a K=1 matmul
                        rdb = rpool.tile([64, 512], BF16, tag="rdb")
                        nc.gpsimd.partition_broadcast(rdb[:], rd_b[:])
                        nc.vector.tensor_mul(
                            aT[hp * 64:(hp + 1) * 64, mo, :],
                            out_ps[hp][0:64, :],
                            rdb[:],
                        )
                aT_prev = aT

            for so in range(4):  # last block's out-proj (nothing to hide it under)
                emit_outproj_chunk(3, so, aT_prev, split_dma=True)

    nc.compile()
    return nc


_NC = None


def _get_nc():
    global _NC
    if _NC is None:
        _NC = build_bass()
    return _NC


def _causal_mask():
    j = np.arange(128)[:, None]
    i = np.arange(128)[None, :]
    return (j <= i).astype(NP_BF16)


def kernel(in_features, Wq, Wk, Wv, Wo):
    global LAST_RESULTS
    nc = _get_nc()

    x = np.asarray(in_features, np.float32)
    Wq = np.asarray(Wq, np.float32)
    Wk = np.asarray(Wk, np.float32)
    Wv = np.asarray(Wv, np.float32)
    Wo = np.asarray(Wo, np.float32)
    mask = _causal_mask()

    in_maps = []
    for c in range(N_CORES):
        b, g = divmod(c, 4)
        cols = slice(g * GC, (g + 1) * GC)
        in_maps.append({
            "xT": np.ascontiguousarray(x[b].T).astype(NP_BF16),
            "wqT": np.ascontiguousarray(Wq.T[:, cols]).astype(NP_BF16),
            "wkT": np.ascontiguousarray(Wk.T[:, cols]).astype(NP_BF16),
            "wvT": np.ascontiguousarray(Wv.T[:, cols]).astype(NP_BF16),
            "woT": np.ascontiguousarray(Wo[:, cols].T).astype(NP_BF16),
            "mask": mask,
        })

    res = bass_utils.run_bass_kernel_spmd(
        nc, in_maps, core_ids=list(range(N_CORES)), trace=TRACE,
    )
    LAST_RESULTS = res
    parts = [res.results[c]["out"].astype(np.float32) for c in range(N_CORES)]
    out = np.stack([
        parts[4 * b] + parts[4 * b + 1] + parts[4 * b + 2] + parts[4 * b + 3]
        for b in range(B)
    ]).astype(np.float32)
    return out
